# revision 9
# baseline (speedup 1.0000x reference)
"""Trainium2 Bass kernel for segment_reduce (mode='average').

Problem: out[b, s] = mean(input[b, ii:jj], axis=0) for s < lengths[b], else 0,
with (ii, jj) = span_indexes[b, s]. Shapes: input [8, 4096, 768] f32,
lengths [8] i32, span_indexes [8, 512, 2] i32.

Primary path (aligned uniform spans: ii = s*w, jj = ii + w, 128 % w == 0,
shared across batches — true for the graded inputs, w = 8): the segment-mean
is a matmul with a periodic block-diagonal weight. A token chunk of 128
tokens covers q = 128/w whole spans, so the only chunks that matter are the
ones whose spans are valid (s < lengths[b]) — roughly half of them for the
graded lengths. The host packs exactly those chunks (from any batch),
load-balanced across the 8 cores, into one [n_slots*128, D] bf16 tensor per
core. On device, slot r of each group of w chunks is matmul'd with a shifted
[128, 128] constant R_r (entries 1/w) accumulating into a [128, D] PSUM span
tile; the tile is then copied to SBUF as bf16 and streamed out. The host
scatters valid rows back into the full [B, S, D] f32 output.

Accuracy: bf16 input + bf16 output rounding gives ~4e-3 max rel err vs the
f32 reference (gate is 2e-2). PSUM accumulation is f32; 1/w is bf16-exact.

Fallback (arbitrary spans): host builds a scaled mask matrix
MT[t, s] = (ii_s <= t < jj_s) * valid_s / (jj_s - ii_s) per batch and the
device does out = MT.T @ x with PSUM accumulation over all 32 token chunks.
"""

import numpy as np

B, T, S, D = 8, 4096, 512, 768
N_CORES = 8
P = 128
K_TILES = T // P  # 32
NT = 384  # matmul moving free-dim tile (<=512 fp32 PSUM)
S_TILES = S // P  # 4

_cache = {}


def _new_bass():
    import concourse.bacc as bacc

    return bacc.Bacc("TRN2", target_bir_lowering=False, debug=False,
                     num_devices=N_CORES)


def _build_packed(w, n_slots):
    """n_slots chunk slots of 128 tokens each, grouped into ceil(n_slots/w)
    PSUM span tiles (slot r of a group covers span-rows [r*q, (r+1)*q) of the
    group's 128-row tile, q = 128/w). Input bf16, output bf16.

    Engine roles: sync+gpsimd alternate input transfer issues (two HW queues
    hide inter-transfer descriptor bubbles), tensor matmuls, vector casts the
    nt0 half of each group's PSUM, scalar casts nt1, gpsimd/scalar issue the
    output DMAs. Transfers are 2 slots with a 1-slot tail so the last
    DMA->matmul->cast->store chain is short."""
    import os

    import concourse.tile as tile
    from concourse import mybir

    bf16 = mybir.dt.bfloat16
    Copy = mybir.ActivationFunctionType.Copy

    q = P // w
    n_full, rem = divmod(n_slots, w)
    group_sizes = [w] * n_full + ([rem] if rem else [])
    n_groups = len(group_sizes)
    n_rows = n_slots * q  # output rows

    nc = _new_bass()
    x_d = nc.dram_tensor("xp", [n_slots * P, D], bf16, kind="ExternalInput")
    r_d = nc.dram_tensor("rmat", [P, w * P], bf16, kind="ExternalInput")
    y_d = nc.dram_tensor("y", [n_rows, D], bf16, kind="ExternalOutput")
    x_ap = x_d.ap()
    y_ap = y_d.ap()

    CPD = int(os.environ.get("SEGRED_CPD", "4"))
    # transfer slot counts: CPD-slot transfers (bigger transfers run closer
    # to peak HBM rate), tapering to 1-slot at the end so the final
    # DMA->matmul->cast->store chain is short
    sizes = []
    remn = n_slots
    while remn > CPD:
        sizes.append(CPD)
        remn -= CPD
    while remn > 1:
        take = max(1, remn // 2)
        sizes.append(take)
        remn -= take
    sizes.append(1)

    def x_chunks(j0, nch):
        # [p, c, d] view of chunk slots [j0, j0+nch)
        return x_ap[j0 * P:(j0 + nch) * P, :].rearrange(
            "(c p) d -> p c d", p=P)

    with tile.TileContext(nc) as tc:
        with (
            tc.tile_pool(name="xp", bufs=max(2, n_slots)) as xp,
            tc.tile_pool(name="pp", bufs=min(4, n_groups), space="PSUM") as pp,
            tc.tile_pool(name="op", bufs=min(3, n_groups)) as op,
            tc.tile_pool(name="sg", bufs=1) as sg,
        ):
            rb = sg.tile([P, w * P], bf16)
            nc.scalar.dma_start(out=rb[:], in_=r_d.ap())
            pst = {}
            ti = 0
            j0 = 0
            for nch in sizes:
                xk = xp.tile([P, nch, D], bf16)
                nc.sync.dma_start(out=xk[:], in_=x_chunks(j0, nch))
                for c in range(nch):
                    j = j0 + c
                    g, r = divmod(j, w)
                    kg = group_sizes[g]
                    if r == 0:
                        pst[g] = [pp.tile([P, NT], mybir.dt.float32,
                                          tag=f"ps{nt}", name=f"ps{nt}_{g}")
                                  for nt in range(D // NT)]
                    for nt in range(D // NT):
                        nc.tensor.matmul(
                            pst[g][nt][:],
                            rb[:, r * P:(r + 1) * P],
                            xk[:, c, nt * NT:(nt + 1) * NT],
                            start=(r == 0), stop=(r == kg - 1))
                    if r == kg - 1:
                        rows = kg * q
                        ybase = g * w * q
                        ot = op.tile([P, D], bf16)
                        nc.vector.tensor_copy(
                            out=ot[0:rows, 0:NT], in_=pst[g][0][0:rows, :])
                        nc.scalar.activation(
                            out=ot[0:rows, NT:D], in_=pst[g][1][0:rows, :],
                            func=Copy)
                        # sync has no input issues left by the last group, so
                        # its out-DMA parallelizes the tail flush chain
                        d0 = nc.sync if g == n_groups - 1 else nc.scalar
                        d0.dma_start(
                            out=y_ap[ybase:ybase + rows, 0:NT],
                            in_=ot[0:rows, 0:NT])
                        nc.scalar.dma_start(
                            out=y_ap[ybase:ybase + rows, NT:D],
                            in_=ot[0:rows, NT:D])
                j0 += nch
                ti += 1
    nc.compile()
    return nc


def _build_general():
    import concourse.tile as tile
    from concourse import mybir

    f32 = mybir.dt.float32

    nc = _new_bass()
    x_d = nc.dram_tensor("xg", [T, D], f32, kind="ExternalInput")
    m_d = nc.dram_tensor("mt", [T, S], f32, kind="ExternalInput")
    y_d = nc.dram_tensor("yg", [S, D], f32, kind="ExternalOutput")
    x_ap = x_d.ap()
    m_ap = m_d.ap()
    y_ap = y_d.ap()

    with tile.TileContext(nc) as tc:
        with (
            tc.tile_pool(name="xp", bufs=3) as xp,
            tc.tile_pool(name="mp", bufs=3) as mp,
            tc.tile_pool(name="op", bufs=2) as op,
            tc.tile_pool(name="pp", bufs=1, space="PSUM") as pp,
        ):
            ps = [[pp.tile([P, NT], f32, tag=f"ps_{st}_{nt}",
                            name=f"ps_{st}_{nt}")
                   for nt in range(D // NT)] for st in range(S_TILES)]
            for k in range(K_TILES):
                xk = xp.tile([P, D], f32)
                nc.sync.dma_start(out=xk[:], in_=x_ap[k * P:(k + 1) * P, :])
                mk = mp.tile([P, S], f32)
                nc.sync.dma_start(out=mk[:], in_=m_ap[k * P:(k + 1) * P, :])
                for st in range(S_TILES):
                    for nt in range(D // NT):
                        nc.tensor.matmul(
                            ps[st][nt][:],
                            mk[:, st * P:(st + 1) * P],
                            xk[:, nt * NT:(nt + 1) * NT],
                            start=(k == 0), stop=(k == K_TILES - 1))
            for st in range(S_TILES):
                ot = op.tile([P, D], f32)
                for nt in range(D // NT):
                    nc.vector.tensor_copy(
                        out=ot[:, nt * NT:(nt + 1) * NT], in_=ps[st][nt][:])
                nc.scalar.dma_start(
                    out=y_ap[st * P:(st + 1) * P, :], in_=ot[:])
    nc.compile()
    return nc


def _detect_aligned(ii, jj):
    """Return span width w if spans are s*w:(s+1)*w for all batches, with
    128 % w == 0 and w a power of two (1/w bf16-exact)."""
    if not (np.all(ii == ii[0]) and np.all(jj == jj[0])):
        return None
    i0, j0 = ii[0], jj[0]
    w = int(j0[0] - i0[0])
    if w < 1 or w > 32 or P % w != 0 or (w & (w - 1)) != 0:
        return None
    if S * w > T:
        return None
    s = np.arange(S, dtype=np.int64)
    if np.any(i0 != s * w) or np.any(j0 != s * w + w):
        return None
    return w


def _rmat(w):
    """[128, w*128] f32: column block r is R_r with R_r[t, s'] = (s' ==
    (128*r + t) // w) / w."""
    rb = np.zeros((P, w * P), dtype=np.float32)
    t = np.arange(P)
    for r in range(w):
        sp = (P * r + t) // w  # in [0, 128)
        rb[t, r * P + sp] = 1.0 / w
    return rb


def _run_spmd(nc, in_maps, **kw):
    from concourse.bass_utils import run_bass_kernel_spmd

    last = None
    for _ in range(3):  # device errors can be transient right after attach
        try:
            return run_bass_kernel_spmd(nc, in_maps, list(range(N_CORES)), **kw)
        except Exception as e:  # noqa: BLE001
            last = e
    raise last


def _prepare(input, lengths, span_indexes):
    x = np.asarray(input, dtype=np.float32)
    lengths = np.asarray(lengths).astype(np.int64)
    si = np.asarray(span_indexes).astype(np.int64)
    assert x.shape == (B, T, D), x.shape
    ii, jj = si[..., 0], si[..., 1]
    valid = (np.arange(S)[None, :] < lengths[:, None])  # [B, S]

    w = _detect_aligned(ii, jj)
    if w is not None:
        import os

        import ml_dtypes

        bf16 = ml_dtypes.bfloat16
        q = P // w  # whole spans per 128-token chunk
        # global list of needed chunks: chunk (b, c) covers spans
        # [c*q, (c+1)*q) of batch b; needed iff c*q < lengths[b]
        chunks = [(b, c) for b in range(B)
                  for c in range(-(-int(lengths[b]) // q))]
        n_slots = -(-len(chunks) // N_CORES)
        key = ("p", w, n_slots, os.environ.get("SEGRED_CPD", "4"))
        if key not in _cache:
            _cache[key] = _build_packed(w, n_slots)
        rb = _rmat(w).astype(bf16)
        in_maps = []
        per_core = []
        for k in range(N_CORES):
            mine = chunks[k * n_slots:(k + 1) * n_slots]
            per_core.append(mine)
            xpack = np.zeros((n_slots * P, D), dtype=bf16)
            for j, (b, c) in enumerate(mine):
                xpack[j * P:(j + 1) * P] = x[b, c * P:(c + 1) * P]
            in_maps.append({"xp": xpack, "rmat": rb})

        def assemble(results):
            out = np.zeros((B, S, D), dtype=np.float32)
            for k in range(N_CORES):
                y = np.asarray(results[k]["y"], dtype=np.float32)
                for j, (b, c) in enumerate(per_core[k]):
                    nv = min(q, int(lengths[b]) - c * q)
                    out[b, c * q:c * q + nv] = y[j * q:j * q + nv]
            return out

        return _cache[key], in_maps, assemble

    if "g" not in _cache:
        _cache["g"] = _build_general()
    n = np.maximum(jj - ii, 1).astype(np.float32)  # [B, S]
    wgt = valid.astype(np.float32) / n  # [B, S]
    t = np.arange(T)[:, None]  # [T, 1]
    in_maps = []
    for b in range(B):
        mt = ((t >= ii[b][None, :]) & (t < jj[b][None, :]))
        mt = mt.astype(np.float32) * wgt[b][None, :]
        in_maps.append({
            "xg": np.ascontiguousarray(x[b]),
            "mt": np.ascontiguousarray(mt),
        })

    def assemble(results):
        return np.ascontiguousarray(
            np.stack([results[b]["yg"] for b in range(B)])
        ).astype(np.float32)

    return _cache["g"], in_maps, assemble


def kernel(input, lengths, span_indexes):
    nc, in_maps, assemble = _prepare(input, lengths, span_indexes)
    res = _run_spmd(nc, in_maps)
    return assemble(res.results)


def run_traced(input, lengths, span_indexes, trace_cores=None):
    """Test-only entry: run with NTFF tracing, return (output, BassKernelResults)."""
    _install_profile_hook()
    nc, in_maps, assemble = _prepare(input, lengths, span_indexes)
    res = _run_spmd(nc, in_maps, trace=True, trace_cores=trace_cores)
    return assemble(res.results), res


def _install_profile_hook():
    import contextlib
    import ctypes
    import sys
    import types

    if "antenv.axon_hooks" in sys.modules:
        return
    lib = ctypes.CDLL("/opt/axon/libaxon_pjrt.so")
    if not hasattr(lib, "axon_start_nrt_profile"):
        hook = None
    else:
        lib.axon_start_nrt_profile.argtypes = [
            ctypes.POINTER(ctypes.c_int64), ctypes.c_size_t]
        lib.axon_start_nrt_profile.restype = ctypes.c_int64
        lib.axon_stop_nrt_profile.argtypes = [ctypes.c_char_p]
        lib.axon_stop_nrt_profile.restype = ctypes.c_int64

        @contextlib.contextmanager
        def hook(output_dir, device_ids):
            import jax

            jax.devices()
            if device_ids:
                ids = (ctypes.c_int64 * len(device_ids))(*device_ids)
                rc = lib.axon_start_nrt_profile(ids, len(device_ids))
            else:
                rc = lib.axon_start_nrt_profile(None, 0)
            if rc != 0:
                raise RuntimeError(f"axon_start_nrt_profile rc={rc}")
            try:
                yield
            finally:
                n = lib.axon_stop_nrt_profile(str(output_dir).encode())
                print(f"profile: {n} ntff file(s) in {output_dir}",
                      file=sys.stderr)

    mod = types.ModuleType("antenv.axon_hooks")
    mod.get_axon_ntff_profile_hook = lambda: hook
    mod.set_axon_ntff_profile_hook = lambda h: None
    sys.modules["antenv.axon_hooks"] = mod

    import concourse.bass_utils as bu

    bu.upload_artifacts = lambda tmpdir: f"local://{tmpdir}"


# revision 11
# speedup vs baseline: 1.0462x; 1.0462x over previous
"""Trainium2 Bass kernel for segment_reduce (mode='average').

Problem: out[b, s] = mean(input[b, ii:jj], axis=0) for s < lengths[b], else 0,
with (ii, jj) = span_indexes[b, s]. Shapes: input [8, 4096, 768] f32,
lengths [8] i32, span_indexes [8, 512, 2] i32.

Primary path (aligned uniform spans: ii = s*w, jj = ii + w, 128 % w == 0,
shared across batches — true for the graded inputs, w = 8): the segment-mean
is a matmul with a periodic block-diagonal weight. A token chunk of 128
tokens covers q = 128/w whole spans, so the only chunks that matter are the
ones whose spans are valid (s < lengths[b]) — roughly half of them for the
graded lengths. The host packs exactly those chunks (from any batch),
load-balanced across the 8 cores, into one [n_slots*128, D] bf16 tensor per
core. On device, slot r of each group of <=w chunks is matmul'd with a
shifted [128, 128] constant R_r (entries 1/w, generated on device with
memset+affine_select — no weight DMA) accumulating into a [128, D] PSUM span
tile; the tile is cast to bf16 SBUF and streamed out. The host scatters
valid rows back into the full [B, S, D] f32 output.

Accuracy: bf16 input + bf16 output rounding gives ~4e-3 max rel err vs the
f32 reference (gate is 2e-2). PSUM accumulation is f32; 1/w is bf16-exact.

Schedule notes (from NTFF traces): ~7us fixed preamble before the first
user instruction and ~2.5us semaphore-reset tail are framework-fixed. All
input transfer issues are hoisted onto the sync queue up-front; group flushes
(vector cast / scalar act-cast, sync/scalar out-DMAs) are aligned to transfer
boundaries so they overlap the stream instead of piling up at the end.

Fallback (arbitrary spans): host builds a scaled mask matrix
MT[t, s] = (ii_s <= t < jj_s) * valid_s / (jj_s - ii_s) per batch and the
device does out = MT.T @ x with PSUM accumulation over all 32 token chunks.
"""

import numpy as np

B, T, S, D = 8, 4096, 512, 768
N_CORES = 8
P = 128
K_TILES = T // P  # 32
NT = 384  # matmul moving free-dim tile (<=512 fp32 PSUM)
S_TILES = S // P  # 4

_cache = {}


def _new_bass():
    import concourse.bacc as bacc

    return bacc.Bacc("TRN2", target_bir_lowering=False, debug=False,
                     num_devices=N_CORES)


def _plan(w, n_slots):
    """Transfer sizes (slots per input DMA) and group sizes (slots per PSUM
    span tile, each <= w). Transfers taper to 1 slot at the end so the final
    DMA->matmul->cast->store chain is short; groups merge transfers in the
    first half and then go one-group-per-transfer so flushes spread out."""
    import os

    cpd = min(int(os.environ.get("SEGRED_CPD", "4")), w)
    sizes = []
    remn = n_slots
    while remn > cpd:
        sizes.append(cpd)
        remn -= cpd
    while remn > 1:
        take = max(1, remn // 2)
        sizes.append(take)
        remn -= take
    sizes.append(1)

    groups = []
    cur = 0
    done = 0
    for t in sizes:
        if cur and (cur + t > w or done >= n_slots // 2):
            groups.append(cur)
            cur = 0
        cur += t
        done += t
    if cur:
        groups.append(cur)
    return sizes, groups


def _build_packed(w, n_slots):
    import concourse.tile as tile
    from concourse import mybir

    bf16 = mybir.dt.bfloat16
    Copy = mybir.ActivationFunctionType.Copy

    q = P // w
    sizes, group_sizes = _plan(w, n_slots)
    n_groups = len(group_sizes)
    gstart = np.concatenate([[0], np.cumsum(group_sizes)])  # slot starts
    n_rows = n_slots * q  # output rows

    nc = _new_bass()
    x_d = nc.dram_tensor("xp", [n_slots * P, D], bf16, kind="ExternalInput")
    y_d = nc.dram_tensor("y", [n_rows, D], bf16, kind="ExternalOutput")
    x_ap = x_d.ap()
    y_ap = y_d.ap()

    def x_chunks(j0, nch):
        # [p, c, d] view of chunk slots [j0, j0+nch)
        return x_ap[j0 * P:(j0 + nch) * P, :].rearrange(
            "(c p) d -> p c d", p=P)

    with tile.TileContext(nc) as tc:
        with (
            tc.tile_pool(name="xp", bufs=max(2, len(sizes))) as xp,
            tc.tile_pool(name="pp", bufs=min(4, n_groups), space="PSUM") as pp,
            tc.tile_pool(name="op", bufs=min(3, n_groups)) as op,
            tc.tile_pool(name="sg", bufs=1) as sg,
        ):
            # rb[t, r*128 + c] = (c == (128*r + t) // w) / w, i.e. the w
            # shifted segment-mean weight blocks, built without any HBM
            # traffic: memset 1/w then zero where t + 128r - wc is outside
            # [0, w-1].
            rb = sg.tile([P, w, P], bf16)
            nc.gpsimd.memset(rb[:], 1.0 / w)
            nc.gpsimd.affine_select(
                out=rb[:], in_=rb[:],
                compare_op=mybir.AluOpType.is_ge, fill=0.0,
                base=0, channel_multiplier=1, pattern=[[P, w], [-w, P]])
            nc.gpsimd.affine_select(
                out=rb[:], in_=rb[:],
                compare_op=mybir.AluOpType.is_gt, fill=0.0,
                base=w, channel_multiplier=-1, pattern=[[-P, w], [w, P]])

            # hoist every input transfer issue; sync does nothing else until
            # the very end, so the stream is never stalled behind compute
            xks = []
            j0 = 0
            for nch in sizes:
                xk = xp.tile([P, nch, D], bf16)
                nc.sync.dma_start(out=xk[:], in_=x_chunks(j0, nch))
                xks.append((j0, nch, xk))
                j0 += nch

            pst = {}
            for j0, nch, xk in xks:
                for c in range(nch):
                    j = j0 + c
                    g = int(np.searchsorted(gstart, j, side="right")) - 1
                    r = j - int(gstart[g])
                    kg = group_sizes[g]
                    if r == 0:
                        pst[g] = [pp.tile([P, NT], mybir.dt.float32,
                                          tag=f"ps{nt}", name=f"ps{nt}_{g}")
                                  for nt in range(D // NT)]
                    for nt in range(D // NT):
                        nc.tensor.matmul(
                            pst[g][nt][:],
                            rb[:, r, :],
                            xk[:, c, nt * NT:(nt + 1) * NT],
                            start=(r == 0), stop=(r == kg - 1))
                    if r == kg - 1:
                        rows = kg * q
                        ybase = int(gstart[g]) * q
                        ot = op.tile([P, D], bf16)
                        nc.vector.tensor_copy(
                            out=ot[0:rows, 0:NT], in_=pst[g][0][0:rows, :])
                        nc.scalar.activation(
                            out=ot[0:rows, NT:D], in_=pst[g][1][0:rows, :],
                            func=Copy)
                        nc.sync.dma_start(
                            out=y_ap[ybase:ybase + rows, 0:NT],
                            in_=ot[0:rows, 0:NT])
                        nc.scalar.dma_start(
                            out=y_ap[ybase:ybase + rows, NT:D],
                            in_=ot[0:rows, NT:D])
    nc.compile()
    return nc


def _build_general():
    import concourse.tile as tile
    from concourse import mybir

    f32 = mybir.dt.float32

    nc = _new_bass()
    x_d = nc.dram_tensor("xg", [T, D], f32, kind="ExternalInput")
    m_d = nc.dram_tensor("mt", [T, S], f32, kind="ExternalInput")
    y_d = nc.dram_tensor("yg", [S, D], f32, kind="ExternalOutput")
    x_ap = x_d.ap()
    m_ap = m_d.ap()
    y_ap = y_d.ap()

    with tile.TileContext(nc) as tc:
        with (
            tc.tile_pool(name="xp", bufs=3) as xp,
            tc.tile_pool(name="mp", bufs=3) as mp,
            tc.tile_pool(name="op", bufs=2) as op,
            tc.tile_pool(name="pp", bufs=1, space="PSUM") as pp,
        ):
            ps = [[pp.tile([P, NT], f32, tag=f"ps_{st}_{nt}",
                            name=f"ps_{st}_{nt}")
                   for nt in range(D // NT)] for st in range(S_TILES)]
            for k in range(K_TILES):
                xk = xp.tile([P, D], f32)
                nc.sync.dma_start(out=xk[:], in_=x_ap[k * P:(k + 1) * P, :])
                mk = mp.tile([P, S], f32)
                nc.sync.dma_start(out=mk[:], in_=m_ap[k * P:(k + 1) * P, :])
                for st in range(S_TILES):
                    for nt in range(D // NT):
                        nc.tensor.matmul(
                            ps[st][nt][:],
                            mk[:, st * P:(st + 1) * P],
                            xk[:, nt * NT:(nt + 1) * NT],
                            start=(k == 0), stop=(k == K_TILES - 1))
            for st in range(S_TILES):
                ot = op.tile([P, D], f32)
                for nt in range(D // NT):
                    nc.vector.tensor_copy(
                        out=ot[:, nt * NT:(nt + 1) * NT], in_=ps[st][nt][:])
                nc.scalar.dma_start(
                    out=y_ap[st * P:(st + 1) * P, :], in_=ot[:])
    nc.compile()
    return nc


def _detect_aligned(ii, jj):
    """Return span width w if spans are s*w:(s+1)*w for all batches, with
    128 % w == 0 and w a power of two (1/w bf16-exact)."""
    if not (np.all(ii == ii[0]) and np.all(jj == jj[0])):
        return None
    i0, j0 = ii[0], jj[0]
    w = int(j0[0] - i0[0])
    if w < 1 or w > 32 or P % w != 0 or (w & (w - 1)) != 0:
        return None
    if S * w > T:
        return None
    s = np.arange(S, dtype=np.int64)
    if np.any(i0 != s * w) or np.any(j0 != s * w + w):
        return None
    return w


def _run_spmd(nc, in_maps, **kw):
    from concourse.bass_utils import run_bass_kernel_spmd

    last = None
    for _ in range(3):  # device errors can be transient right after attach
        try:
            return run_bass_kernel_spmd(nc, in_maps, list(range(N_CORES)), **kw)
        except Exception as e:  # noqa: BLE001
            last = e
    raise last


def _prepare(input, lengths, span_indexes):
    import os

    x = np.asarray(input, dtype=np.float32)
    lengths = np.asarray(lengths).astype(np.int64)
    si = np.asarray(span_indexes).astype(np.int64)
    assert x.shape == (B, T, D), x.shape
    ii, jj = si[..., 0], si[..., 1]
    valid = (np.arange(S)[None, :] < lengths[:, None])  # [B, S]

    w = _detect_aligned(ii, jj)
    if w is not None:
        import ml_dtypes

        bf16 = ml_dtypes.bfloat16
        q = P // w  # whole spans per 128-token chunk
        # global list of needed chunks: chunk (b, c) covers spans
        # [c*q, (c+1)*q) of batch b; needed iff c*q < lengths[b]
        chunks = [(b, c) for b in range(B)
                  for c in range(-(-int(lengths[b]) // q))]
        n_slots = -(-len(chunks) // N_CORES)
        key = ("p", w, n_slots, os.environ.get("SEGRED_CPD", "4"))
        if key not in _cache:
            _cache[key] = _build_packed(w, n_slots)
        _, group_sizes = _plan(w, n_slots)
        gstart = np.concatenate([[0], np.cumsum(group_sizes)])
        in_maps = []
        per_core = []
        for k in range(N_CORES):
            mine = chunks[k * n_slots:(k + 1) * n_slots]
            per_core.append(mine)
            xpack = np.zeros((n_slots * P, D), dtype=bf16)
            for j, (b, c) in enumerate(mine):
                xpack[j * P:(j + 1) * P] = x[b, c * P:(c + 1) * P]
            in_maps.append({"xp": xpack})

        def assemble(results):
            out = np.zeros((B, S, D), dtype=np.float32)
            for k in range(N_CORES):
                y = np.asarray(results[k]["y"], dtype=np.float32)
                for j, (b, c) in enumerate(per_core[k]):
                    g = int(np.searchsorted(gstart, j, side="right")) - 1
                    row = int(gstart[g]) * q + (j - int(gstart[g])) * q
                    nv = min(q, int(lengths[b]) - c * q)
                    out[b, c * q:c * q + nv] = y[row:row + nv]
            return out

        return _cache[key], in_maps, assemble

    if "g" not in _cache:
        _cache["g"] = _build_general()
    n = np.maximum(jj - ii, 1).astype(np.float32)  # [B, S]
    wgt = valid.astype(np.float32) / n  # [B, S]
    t = np.arange(T)[:, None]  # [T, 1]
    in_maps = []
    for b in range(B):
        mt = ((t >= ii[b][None, :]) & (t < jj[b][None, :]))
        mt = mt.astype(np.float32) * wgt[b][None, :]
        in_maps.append({
            "xg": np.ascontiguousarray(x[b]),
            "mt": np.ascontiguousarray(mt),
        })

    def assemble(results):
        return np.ascontiguousarray(
            np.stack([results[b]["yg"] for b in range(B)])
        ).astype(np.float32)

    return _cache["g"], in_maps, assemble


def kernel(input, lengths, span_indexes):
    nc, in_maps, assemble = _prepare(input, lengths, span_indexes)
    res = _run_spmd(nc, in_maps)
    return assemble(res.results)


def run_traced(input, lengths, span_indexes, trace_cores=None):
    """Test-only entry: run with NTFF tracing, return (output, BassKernelResults)."""
    _install_profile_hook()
    nc, in_maps, assemble = _prepare(input, lengths, span_indexes)
    res = _run_spmd(nc, in_maps, trace=True, trace_cores=trace_cores)
    return assemble(res.results), res


def _install_profile_hook():
    import contextlib
    import ctypes
    import sys
    import types

    if "antenv.axon_hooks" in sys.modules:
        return
    lib = ctypes.CDLL("/opt/axon/libaxon_pjrt.so")
    if not hasattr(lib, "axon_start_nrt_profile"):
        hook = None
    else:
        lib.axon_start_nrt_profile.argtypes = [
            ctypes.POINTER(ctypes.c_int64), ctypes.c_size_t]
        lib.axon_start_nrt_profile.restype = ctypes.c_int64
        lib.axon_stop_nrt_profile.argtypes = [ctypes.c_char_p]
        lib.axon_stop_nrt_profile.restype = ctypes.c_int64

        @contextlib.contextmanager
        def hook(output_dir, device_ids):
            import jax

            jax.devices()
            if device_ids:
                ids = (ctypes.c_int64 * len(device_ids))(*device_ids)
                rc = lib.axon_start_nrt_profile(ids, len(device_ids))
            else:
                rc = lib.axon_start_nrt_profile(None, 0)
            if rc != 0:
                raise RuntimeError(f"axon_start_nrt_profile rc={rc}")
            try:
                yield
            finally:
                n = lib.axon_stop_nrt_profile(str(output_dir).encode())
                print(f"profile: {n} ntff file(s) in {output_dir}",
                      file=sys.stderr)

    mod = types.ModuleType("antenv.axon_hooks")
    mod.get_axon_ntff_profile_hook = lambda: hook
    mod.set_axon_ntff_profile_hook = lambda h: None
    sys.modules["antenv.axon_hooks"] = mod

    import concourse.bass_utils as bu

    bu.upload_artifacts = lambda tmpdir: f"local://{tmpdir}"
